# revision 6
# baseline (speedup 1.0000x reference)
"""Trainium2 Bass kernel for a 2-layer bidirectional GRU.

Problem shapes (hardcoded): T=512, B=64, IN=512, H=512, L=2.

Sharding: 8 NeuronCores = 4 batch quarters x 2 directions (16 batch rows per
core, one scan direction per core). Each layer runs as one SPMD launch; the
bwd direction is realized by passing time-reversed data, so all 8 cores run
the same program. The host shuffles/concats between the two layer launches
and assembles the final outputs.

On-chip layout is "transposed": gate pre-activations and the hidden state
live as [gate/hidden position (128 partitions), batch (free)] tiles so the
serial per-step elementwise chain runs on 128-partition tiles. The recurrent
matmul keeps W chunks stationary (lhsT) and streams h^T (16 cols). The input
projection x @ W_ih is computed in bulk per 32-step window, amortizing
weight loads over 512-column moving operands.
"""

import numpy as np

import concourse.bacc as bacc
import concourse.bass as bass
from concourse import mybir
from concourse.tile import TileContext
from concourse.bass_utils import run_bass_kernel_spmd

import ml_dtypes

P = 128
B = 16          # batch rows per core
H = 512
G = 3 * H       # gate width 1536
KR = H // P     # 4 recurrent k-chunks
MCH = G // P    # 12 gate m-chunks
WIN = 32        # time steps per projection window
N_CORES = 8

F32 = mybir.dt.float32
F32R = mybir.dt.float32r
BF16 = mybir.dt.bfloat16

# matmul precision mode: "f32r" (fp32 storage, reduced-precision PE pass),
# "bf16" (bf16 weights/state), or "f32" (full precision, slow)
MM_MODE = "f32r"

# set by test harnesses to capture per-layer HW exec time / traces
TRACE = False
LAYER_EXEC_NS = {}
TRACE_DIRS = {}

_SIG = mybir.ActivationFunctionType.Sigmoid
_TANH = mybir.ActivationFunctionType.Tanh
_IDENT = mybir.ActivationFunctionType.Identity
_MUL = mybir.AluOpType.mult
_ADD = mybir.AluOpType.add
_SUB = mybir.AluOpType.subtract


def build_layer(T, K_in, mode):
    """Build the SPMD single-layer kernel. Returns finalized nc."""
    KI = K_in // P
    NW = T // WIN
    NT = WIN * B    # moving cols per window (512)

    if mode == "f32r":
        wdt = F32R      # weights dram/sbuf dtype (bits are f32)
        xdt = F32R      # xT dtype
        hdt = F32R      # h state dtype (DVE rounds on write)
        rhs_cast = None
    elif mode == "bf16":
        wdt = BF16
        xdt = BF16
        hdt = BF16
        rhs_cast = None
    elif mode == "f32":
        wdt = F32
        xdt = F32
        hdt = F32
        rhs_cast = None
    else:
        raise ValueError(mode)

    nc = bacc.Bacc(None)

    xT = nc.declare_dram_parameter("xT", [KI, P, T * B], xdt, isOutput=False)
    wih = nc.declare_dram_parameter("wih", [KI, P, G], wdt, isOutput=False)
    wcat = nc.declare_dram_parameter("wcat", [KR, P, G], wdt, isOutput=False)
    bih = nc.declare_dram_parameter("bih", [P, MCH], F32, isOutput=False)
    bnh = nc.declare_dram_parameter("bnh", [P, KR * B], F32, isOutput=False)
    outT = nc.declare_dram_parameter("outT", [KR, P, T, B], F32, isOutput=True)
    hlast = nc.declare_dram_parameter("hlast", [P, KR * B], F32, isOutput=True)

    # DRAM views
    xT_w = xT.ap().rearrange("k p (w n) -> k p w n", n=NT)        # per-window cols
    out_v = outT.ap().rearrange("k p t b -> p k t b")

    with TileContext(nc) as tc:
        with (
            tc.tile_pool(name="singles", bufs=1) as singles,
            tc.tile_pool(name="xin", bufs=2) as xin_pool,
            tc.tile_pool(name="xproj", bufs=2) as xproj_pool,
            tc.tile_pool(name="hstate", bufs=3) as h_pool,
            tc.tile_pool(name="gates", bufs=2) as g_pool,
            tc.tile_pool(name="psb", bufs=2, space="PSUM") as ps_bulk,
            tc.tile_pool(name="psr", bufs=2, space="PSUM") as ps_r,
            tc.tile_pool(name="psz", bufs=2, space="PSUM") as ps_z,
            tc.tile_pool(name="psn", bufs=2, space="PSUM") as ps_n,
        ):
            # ---- load constants ----
            wih_sb = singles.tile([P, KI, G], wdt)
            nc.sync.dma_start(out=wih_sb[:], in_=wih.ap().rearrange("k p g -> p k g"))
            wcat_sb = singles.tile([P, KR, G], wdt)
            nc.sync.dma_start(out=wcat_sb[:], in_=wcat.ap().rearrange("k p g -> p k g"))
            bih_sb = singles.tile([P, MCH], F32)
            nc.sync.dma_start(out=bih_sb[:], in_=bih.ap())
            bnh_sb = singles.tile([P, KR * B], F32)
            nc.sync.dma_start(out=bnh_sb[:], in_=bnh.ap())

            def load_xin(w):
                t = xin_pool.tile([P, KI, NT], xdt, tag="xin")
                nc.sync.dma_start(
                    out=t[:], in_=xT_w[:, :, w, :].rearrange("k p n -> p k n")
                )
                return t

            def project(w, xin_t):
                """x_proj for window w -> sbuf tile [P, MCH, WIN, B] (f32)."""
                xp = xproj_pool.tile([P, WIN, MCH, B], F32, tag="xproj")
                for m in range(MCH):
                    ps = ps_bulk.tile([P, NT], F32, tag="psb")
                    for k in range(KI):
                        nc.tensor.matmul(
                            ps[:],
                            lhsT=wih_sb[:, k, m * P:(m + 1) * P],
                            rhs=xin_t[:, k, :],
                            start=(k == 0),
                            stop=(k == KI - 1),
                        )
                    nc.scalar.activation(
                        out=xp[:, :, m, :],
                        in_=ps.rearrange("p (w b) -> p w b", b=B),
                        func=_IDENT, bias=bih_sb[:, m:m + 1], scale=1.0,
                    )
                return xp

            # ---- prologue ----
            h = h_pool.tile([P, KR * B], hdt, tag="h")
            if hdt == F32R:
                # memset can't target f32r; produce rounded zeros via DVE
                nc.vector.tensor_scalar_mul(h[:], bnh_sb[:], 0.0)
            else:
                nc.vector.memset(h[:], 0.0)

            xin_t = load_xin(0)
            xp_cur = project(0, xin_t)
            xp_next = None

            for w in range(NW):
                if w + 1 < NW:
                    xin_t = load_xin(w + 1)
                    xp_next = project(w + 1, xin_t)

                for j in range(WIN):
                    s = w * WIN + j
                    rhs = h[:] if rhs_cast is None else h[:].bitcast(rhs_cast)
                    psr = ps_r.tile([P, KR * B], F32, tag="psr")
                    psz = ps_z.tile([P, KR * B], F32, tag="psz")
                    psn = ps_n.tile([P, KR * B], F32, tag="psn")
                    for ps, g0 in ((psr, 0), (psz, 1), (psn, 2)):
                        for m in range(KR):
                            c0 = g0 * H + m * P
                            for k in range(KR):
                                nc.tensor.matmul(
                                    ps[:, m * B:(m + 1) * B],
                                    lhsT=wcat_sb[:, k, c0:c0 + P],
                                    rhs=rhs[:, k * B:(k + 1) * B],
                                    start=(k == 0),
                                    stop=(k == KR - 1),
                                )

                    xp_r = xp_cur[:, j, 0:KR, :].rearrange("p m b -> p (m b)")
                    xp_z = xp_cur[:, j, KR:2 * KR, :].rearrange("p m b -> p (m b)")
                    xp_n = xp_cur[:, j, 2 * KR:3 * KR, :].rearrange("p m b -> p (m b)")

                    r1 = g_pool.tile([P, KR * B], F32, tag="r1")
                    nc.vector.tensor_add(r1[:], psr[:], xp_r)
                    r = g_pool.tile([P, KR * B], F32, tag="r")
                    nc.scalar.activation(out=r[:], in_=r1[:], func=_SIG)

                    z1 = g_pool.tile([P, KR * B], F32, tag="z1")
                    nc.vector.tensor_add(z1[:], psz[:], xp_z)
                    z = g_pool.tile([P, KR * B], F32, tag="z")
                    nc.scalar.activation(out=z[:], in_=z1[:], func=_SIG)

                    # zh = z * h   (hidden under MM stream)
                    zh = g_pool.tile([P, KR * B], F32, tag="zh")
                    nc.vector.tensor_mul(zh[:], z[:], h[:])

                    # u = n_x + r * b_nh   (hidden)
                    rb = g_pool.tile([P, KR * B], F32, tag="rb")
                    nc.vector.tensor_mul(rb[:], r[:], bnh_sb[:])
                    u = g_pool.tile([P, KR * B], F32, tag="u")
                    nc.vector.tensor_add(u[:], rb[:], xp_n)

                    # tail: t2 = r*psn + u ; n = tanh(t2)
                    t1 = g_pool.tile([P, KR * B], F32, tag="t1")
                    nc.vector.tensor_mul(t1[:], r[:], psn[:])
                    t2 = g_pool.tile([P, KR * B], F32, tag="t2")
                    nc.vector.tensor_add(t2[:], t1[:], u[:])
                    n = g_pool.tile([P, KR * B], F32, tag="n")
                    nc.scalar.activation(out=n[:], in_=t2[:], func=_TANH)

                    # h_new = (1-z)*n + z*h = -( (z-1)*n ) + zh
                    g2n = g_pool.tile([P, KR * B], F32, tag="g2n")
                    nc.vector.scalar_tensor_tensor(
                        out=g2n[:], in0=z[:], scalar=1.0, in1=n[:],
                        op0=_SUB, op1=_MUL,
                    )
                    h_new = h_pool.tile([P, KR * B], hdt, tag="h")
                    nc.vector.scalar_tensor_tensor(
                        out=h_new[:], in0=g2n[:], scalar=-1.0, in1=zh[:],
                        op0=_MUL, op1=_ADD,
                    )

                    # store h_new -> outT[:, :, s, :]
                    if hdt == F32R:
                        st_src = h_new[:].bitcast(F32).rearrange(
                            "p (k b) -> p k b", b=B)
                        nc.sync.dma_start(out=out_v[:, :, s, :], in_=st_src)
                        h = h_new
                    else:
                        # keep an f32 copy for output
                        h32 = g_pool.tile([P, KR * B], F32, tag="h32")
                        nc.vector.tensor_copy(h32[:], h_new[:])
                        st_src = h32.rearrange("p (k b) -> p k b", b=B)
                        nc.sync.dma_start(out=out_v[:, :, s, :], in_=st_src)
                        h = h_new

                xp_cur = xp_next

            # final hidden state
            fin = g_pool.tile([P, KR * B], F32, tag="fin")
            nc.vector.tensor_copy(fin[:], h[:])
            nc.sync.dma_start(out=hlast.ap(), in_=fin[:])

    nc.finalize()
    return nc


# ---------------------------------------------------------------------------
# host orchestration
# ---------------------------------------------------------------------------

_BUILD_CACHE = {}


def _get_layer(T, K_in, mode):
    key = (T, K_in, mode)
    if key not in _BUILD_CACHE:
        _BUILD_CACHE[key] = build_layer(T, K_in, mode)
    return _BUILD_CACHE[key]


def _np_cast(a, dt):
    if dt == BF16:
        return np.asarray(a, np.float32).astype(ml_dtypes.bfloat16)
    return np.ascontiguousarray(np.asarray(a, np.float32))


def _run_layer(layer_in, cells_fwd, cells_bwd, mode, T):
    """layer_in: (T, 64, K_in) f32. cells_*: param dicts.
    Returns (out (T,64,2H) f32, hf (64,H), hb (64,H))."""
    K_in = layer_in.shape[2]
    KI = K_in // P
    nc = _get_layer(T, K_in, mode)
    mmdt = {"f32r": F32R, "bf16": BF16, "f32": F32}[mode]
    xdt = mmdt

    in_maps = []
    for core in range(N_CORES):
        q, d = core % 4, core // 4
        cell = cells_fwd if d == 0 else cells_bwd
        xc = layer_in[:, q * B:(q + 1) * B, :]
        if d == 1:
            xc = xc[::-1]
        xTc = np.ascontiguousarray(xc.transpose(2, 0, 1)).reshape(KI, P, T * B)
        W_ih = np.asarray(cell["W_ih"], np.float32)       # (K_in, 3H)
        W_rzh = np.asarray(cell["W_rzh"], np.float32)     # (H, 2H)
        W_nh = np.asarray(cell["W_nh"], np.float32)       # (H, H)
        b_ih = np.asarray(cell["b_ih"], np.float32)       # (3H,)
        b_nh = np.asarray(cell["b_nh"], np.float32)       # (H,)
        wcat = np.concatenate([W_rzh, W_nh], axis=1)      # (H, 3H)
        in_maps.append({
            "xT": _np_cast(xTc, xdt),
            "wih": _np_cast(W_ih.reshape(KI, P, G), mmdt),
            "wcat": _np_cast(wcat.reshape(KR, P, G), mmdt),
            "bih": np.ascontiguousarray(b_ih.reshape(MCH, P).T),
            "bnh": np.ascontiguousarray(
                np.repeat(b_nh.reshape(KR, P).T[:, :, None], B, axis=2)
                .reshape(P, KR * B)),
        })

    if TRACE:
        import tempfile
        td = tempfile.mkdtemp(prefix=f"gru_l{K_in}_")
        res = run_bass_kernel_spmd(nc, in_maps, list(range(N_CORES)),
                                   trace=True, tmpdir=td)
        LAYER_EXEC_NS[(T, K_in, mode)] = res.exec_time_ns
        TRACE_DIRS[(T, K_in, mode)] = td
    else:
        res = run_bass_kernel_spmd(nc, in_maps, list(range(N_CORES)))

    out = np.empty((T, 64, 2 * H), np.float32)
    hf = np.empty((64, H), np.float32)
    hb = np.empty((64, H), np.float32)
    for core in range(N_CORES):
        q, d = core % 4, core // 4
        o = res.results[core]["outT"]          # (KR, P, T, B)
        hl = res.results[core]["hlast"]        # (P, KR*B)
        o = o.transpose(2, 3, 0, 1).reshape(T, B, H)     # (t, b, h)
        if d == 1:
            o = o[::-1]
        out[:, q * B:(q + 1) * B, d * H:(d + 1) * H] = o
        hlast = hl.reshape(P, KR, B).transpose(2, 1, 0).reshape(B, H)
        (hf if d == 0 else hb)[q * B:(q + 1) * B] = hlast
    return out, hf, hb


def kernel(x, params):
    T = 512
    x = np.asarray(x, np.float32)
    mode = MM_MODE

    layer_in = x
    finals = []
    for lp in params:
        out, hf, hb = _run_layer(layer_in, lp["fwd"], lp["bwd"], mode, T)
        finals.extend([hf, hb])
        layer_in = out
    hidden = np.stack(finals)
    return layer_in, hidden


# revision 7
# speedup vs baseline: 2.9959x; 2.9959x over previous
"""Trainium2 Bass kernel for a 2-layer bidirectional GRU.

Problem shapes (hardcoded): T=512, B=64, IN=512, H=512, L=2.

Sharding: 8 NeuronCores = 4 batch quarters x 2 directions (16 batch rows per
core, one scan direction per core). Each layer runs as one SPMD launch; the
bwd direction is realized by passing time-reversed data, so all 8 cores run
the same program. The host shuffles/concats between the two layer launches
and assembles the final outputs.

On-chip layout is "transposed": gate pre-activations and the hidden state
live as [gate/hidden position (128 partitions), batch (free)] tiles so the
serial per-step elementwise chain runs on 128-partition tiles. The recurrent
matmul keeps W chunks stationary (lhsT) and streams h^T (16 cols). The input
projection x @ W_ih is computed in bulk per 32-step window, amortizing
weight loads over 512-column moving operands.
"""

import numpy as np

import concourse.bacc as bacc
import concourse.bass as bass
from concourse import mybir
from concourse.tile import TileContext
from concourse.bass_utils import run_bass_kernel_spmd

import ml_dtypes

P = 128
B = 16          # batch rows per core
H = 512
G = 3 * H       # gate width 1536
KR = H // P     # 4 recurrent k-chunks
MCH = G // P    # 12 gate m-chunks
WIN = 32        # time steps per projection window
N_CORES = 8

F32 = mybir.dt.float32
F32R = mybir.dt.float32r
BF16 = mybir.dt.bfloat16

# matmul precision mode: "f32r" (fp32 storage, reduced-precision PE pass),
# "bf16" (bf16 weights/state), or "f32" (full precision, slow)
MM_MODE = "bf16"

# set by test harnesses to capture per-layer HW exec time / traces
TRACE = False
LAYER_EXEC_NS = {}
TRACE_DIRS = {}

_SIG = mybir.ActivationFunctionType.Sigmoid
_TANH = mybir.ActivationFunctionType.Tanh
_IDENT = mybir.ActivationFunctionType.Identity
_MUL = mybir.AluOpType.mult
_ADD = mybir.AluOpType.add
_SUB = mybir.AluOpType.subtract


def build_layer(T, K_in, mode):
    """Build the SPMD single-layer kernel. Returns finalized nc."""
    KI = K_in // P
    NW = T // WIN
    NT = WIN * B    # moving cols per window (512)

    if mode == "f32r":
        wdt = F32R      # weights dram/sbuf dtype (bits are f32)
        xdt = F32R      # xT dtype
        hdt = F32R      # h state dtype (DVE rounds on write)
        rhs_cast = None
    elif mode == "bf16":
        wdt = BF16
        xdt = BF16
        hdt = BF16
        rhs_cast = None
    elif mode == "f32":
        wdt = F32
        xdt = F32
        hdt = F32
        rhs_cast = None
    else:
        raise ValueError(mode)

    nc = bacc.Bacc(None)

    xT = nc.declare_dram_parameter("xT", [KI, P, T * B], xdt, isOutput=False)
    wih = nc.declare_dram_parameter("wih", [KI, P, G], wdt, isOutput=False)
    wcat = nc.declare_dram_parameter("wcat", [KR, P, G], wdt, isOutput=False)
    bih = nc.declare_dram_parameter("bih", [P, MCH], F32, isOutput=False)
    bnh = nc.declare_dram_parameter("bnh", [P, KR * B], F32, isOutput=False)
    outT = nc.declare_dram_parameter("outT", [KR, P, T, B], F32, isOutput=True)
    hlast = nc.declare_dram_parameter("hlast", [P, KR * B], F32, isOutput=True)

    # DRAM views
    xT_w = xT.ap().rearrange("k p (w n) -> k p w n", n=NT)        # per-window cols
    out_v = outT.ap().rearrange("k p t b -> p k t b")

    with TileContext(nc) as tc:
        with (
            tc.tile_pool(name="singles", bufs=1) as singles,
            tc.tile_pool(name="xin", bufs=2) as xin_pool,
            tc.tile_pool(name="xproj", bufs=2) as xproj_pool,
            tc.tile_pool(name="hstate", bufs=3) as h_pool,
            tc.tile_pool(name="gates", bufs=2) as g_pool,
            tc.tile_pool(name="psb", bufs=2, space="PSUM") as ps_bulk,
            tc.tile_pool(name="psr", bufs=2, space="PSUM") as ps_r,
            tc.tile_pool(name="psz", bufs=2, space="PSUM") as ps_z,
            tc.tile_pool(name="psn", bufs=2, space="PSUM") as ps_n,
        ):
            # ---- load constants ----
            wih_sb = singles.tile([P, KI, G], wdt)
            nc.sync.dma_start(out=wih_sb[:], in_=wih.ap().rearrange("k p g -> p k g"))
            wcat_sb = singles.tile([P, KR, G], wdt)
            nc.sync.dma_start(out=wcat_sb[:], in_=wcat.ap().rearrange("k p g -> p k g"))
            bih_sb = singles.tile([P, MCH], F32)
            nc.sync.dma_start(out=bih_sb[:], in_=bih.ap())
            bnh_sb = singles.tile([P, KR * B], F32)
            nc.sync.dma_start(out=bnh_sb[:], in_=bnh.ap())

            def load_xin(w):
                t = xin_pool.tile([P, KI, NT], xdt, tag="xin")
                nc.sync.dma_start(
                    out=t[:], in_=xT_w[:, :, w, :].rearrange("k p n -> p k n")
                )
                return t

            def project(w, xin_t):
                """x_proj for window w -> sbuf tile [P, MCH, WIN, B] (f32)."""
                xp = xproj_pool.tile([P, WIN, MCH, B], F32, tag="xproj")
                for m in range(MCH):
                    ps = ps_bulk.tile([P, NT], F32, tag="psb")
                    for k in range(KI):
                        nc.tensor.matmul(
                            ps[:],
                            lhsT=wih_sb[:, k, m * P:(m + 1) * P],
                            rhs=xin_t[:, k, :],
                            start=(k == 0),
                            stop=(k == KI - 1),
                        )
                    nc.scalar.activation(
                        out=xp[:, :, m, :],
                        in_=ps.rearrange("p (w b) -> p w b", b=B),
                        func=_IDENT, bias=bih_sb[:, m:m + 1], scale=1.0,
                    )
                return xp

            # ---- prologue ----
            h = h_pool.tile([P, KR * B], hdt, tag="h")
            if hdt == F32R:
                # memset can't target f32r; produce rounded zeros via DVE
                nc.vector.tensor_scalar_mul(h[:], bnh_sb[:], 0.0)
            else:
                nc.vector.memset(h[:], 0.0)

            xin_t = load_xin(0)
            xp_cur = project(0, xin_t)
            xp_next = None

            for w in range(NW):
                if w + 1 < NW:
                    xin_t = load_xin(w + 1)
                    xp_next = project(w + 1, xin_t)

                for j in range(WIN):
                    s = w * WIN + j
                    rhs = h[:] if rhs_cast is None else h[:].bitcast(rhs_cast)
                    psr = ps_r.tile([P, KR * B], F32, tag="psr")
                    psz = ps_z.tile([P, KR * B], F32, tag="psz")
                    psn = ps_n.tile([P, KR * B], F32, tag="psn")
                    for ps, g0 in ((psr, 0), (psz, 1), (psn, 2)):
                        for m in range(KR):
                            c0 = g0 * H + m * P
                            for k in range(KR):
                                nc.tensor.matmul(
                                    ps[:, m * B:(m + 1) * B],
                                    lhsT=wcat_sb[:, k, c0:c0 + P],
                                    rhs=rhs[:, k * B:(k + 1) * B],
                                    start=(k == 0),
                                    stop=(k == KR - 1),
                                )

                    xp_r = xp_cur[:, j, 0:KR, :].rearrange("p m b -> p (m b)")
                    xp_z = xp_cur[:, j, KR:2 * KR, :].rearrange("p m b -> p (m b)")
                    xp_n = xp_cur[:, j, 2 * KR:3 * KR, :].rearrange("p m b -> p (m b)")

                    r1 = g_pool.tile([P, KR * B], F32, tag="r1")
                    nc.vector.tensor_add(r1[:], psr[:], xp_r)
                    r = g_pool.tile([P, KR * B], F32, tag="r")
                    nc.scalar.activation(out=r[:], in_=r1[:], func=_SIG)

                    z1 = g_pool.tile([P, KR * B], F32, tag="z1")
                    nc.vector.tensor_add(z1[:], psz[:], xp_z)
                    z = g_pool.tile([P, KR * B], F32, tag="z")
                    nc.scalar.activation(out=z[:], in_=z1[:], func=_SIG)

                    # zh = z * h   (hidden under MM stream)
                    zh = g_pool.tile([P, KR * B], F32, tag="zh")
                    nc.vector.tensor_mul(zh[:], z[:], h[:])

                    # u = n_x + r * b_nh   (hidden)
                    rb = g_pool.tile([P, KR * B], F32, tag="rb")
                    nc.vector.tensor_mul(rb[:], r[:], bnh_sb[:])
                    u = g_pool.tile([P, KR * B], F32, tag="u")
                    nc.vector.tensor_add(u[:], rb[:], xp_n)

                    # tail: t2 = r*psn + u ; n = tanh(t2)
                    t1 = g_pool.tile([P, KR * B], F32, tag="t1")
                    nc.vector.tensor_mul(t1[:], r[:], psn[:])
                    t2 = g_pool.tile([P, KR * B], F32, tag="t2")
                    nc.vector.tensor_add(t2[:], t1[:], u[:])
                    n = g_pool.tile([P, KR * B], F32, tag="n")
                    nc.scalar.activation(out=n[:], in_=t2[:], func=_TANH)

                    # h_new = (1-z)*n + z*h = -( (z-1)*n ) + zh
                    g2n = g_pool.tile([P, KR * B], F32, tag="g2n")
                    nc.vector.scalar_tensor_tensor(
                        out=g2n[:], in0=z[:], scalar=1.0, in1=n[:],
                        op0=_SUB, op1=_MUL,
                    )
                    h_new = h_pool.tile([P, KR * B], hdt, tag="h")
                    nc.vector.scalar_tensor_tensor(
                        out=h_new[:], in0=g2n[:], scalar=-1.0, in1=zh[:],
                        op0=_MUL, op1=_ADD,
                    )

                    # store h_new -> outT[:, :, s, :]
                    if hdt == F32R:
                        st_src = h_new[:].bitcast(F32).rearrange(
                            "p (k b) -> p k b", b=B)
                        nc.sync.dma_start(out=out_v[:, :, s, :], in_=st_src)
                        h = h_new
                    else:
                        # keep an f32 copy for output
                        h32 = g_pool.tile([P, KR * B], F32, tag="h32")
                        nc.vector.tensor_copy(h32[:], h_new[:])
                        st_src = h32.rearrange("p (k b) -> p k b", b=B)
                        nc.sync.dma_start(out=out_v[:, :, s, :], in_=st_src)
                        h = h_new

                xp_cur = xp_next

            # final hidden state
            fin = g_pool.tile([P, KR * B], F32, tag="fin")
            nc.vector.tensor_copy(fin[:], h[:])
            nc.sync.dma_start(out=hlast.ap(), in_=fin[:])

    nc.finalize()
    return nc


# ---------------------------------------------------------------------------
# host orchestration
# ---------------------------------------------------------------------------

_BUILD_CACHE = {}


def _get_layer(T, K_in, mode):
    key = (T, K_in, mode)
    if key not in _BUILD_CACHE:
        _BUILD_CACHE[key] = build_layer(T, K_in, mode)
    return _BUILD_CACHE[key]


def _np_cast(a, dt):
    if dt == BF16:
        return np.asarray(a, np.float32).astype(ml_dtypes.bfloat16)
    return np.ascontiguousarray(np.asarray(a, np.float32))


def _run_layer(layer_in, cells_fwd, cells_bwd, mode, T):
    """layer_in: (T, 64, K_in) f32. cells_*: param dicts.
    Returns (out (T,64,2H) f32, hf (64,H), hb (64,H))."""
    K_in = layer_in.shape[2]
    KI = K_in // P
    nc = _get_layer(T, K_in, mode)
    mmdt = {"f32r": F32R, "bf16": BF16, "f32": F32}[mode]
    xdt = mmdt

    in_maps = []
    for core in range(N_CORES):
        q, d = core % 4, core // 4
        cell = cells_fwd if d == 0 else cells_bwd
        xc = layer_in[:, q * B:(q + 1) * B, :]
        if d == 1:
            xc = xc[::-1]
        xTc = np.ascontiguousarray(xc.transpose(2, 0, 1)).reshape(KI, P, T * B)
        W_ih = np.asarray(cell["W_ih"], np.float32)       # (K_in, 3H)
        W_rzh = np.asarray(cell["W_rzh"], np.float32)     # (H, 2H)
        W_nh = np.asarray(cell["W_nh"], np.float32)       # (H, H)
        b_ih = np.asarray(cell["b_ih"], np.float32)       # (3H,)
        b_nh = np.asarray(cell["b_nh"], np.float32)       # (H,)
        wcat = np.concatenate([W_rzh, W_nh], axis=1)      # (H, 3H)
        in_maps.append({
            "xT": _np_cast(xTc, xdt),
            "wih": _np_cast(W_ih.reshape(KI, P, G), mmdt),
            "wcat": _np_cast(wcat.reshape(KR, P, G), mmdt),
            "bih": np.ascontiguousarray(b_ih.reshape(MCH, P).T),
            "bnh": np.ascontiguousarray(
                np.repeat(b_nh.reshape(KR, P).T[:, :, None], B, axis=2)
                .reshape(P, KR * B)),
        })

    if TRACE:
        import tempfile
        td = tempfile.mkdtemp(prefix=f"gru_l{K_in}_")
        res = run_bass_kernel_spmd(nc, in_maps, list(range(N_CORES)),
                                   trace=True, tmpdir=td)
        LAYER_EXEC_NS[(T, K_in, mode)] = res.exec_time_ns
        TRACE_DIRS[(T, K_in, mode)] = td
    else:
        res = run_bass_kernel_spmd(nc, in_maps, list(range(N_CORES)))

    out = np.empty((T, 64, 2 * H), np.float32)
    hf = np.empty((64, H), np.float32)
    hb = np.empty((64, H), np.float32)
    for core in range(N_CORES):
        q, d = core % 4, core // 4
        o = res.results[core]["outT"]          # (KR, P, T, B)
        hl = res.results[core]["hlast"]        # (P, KR*B)
        o = o.transpose(2, 3, 0, 1).reshape(T, B, H)     # (t, b, h)
        if d == 1:
            o = o[::-1]
        out[:, q * B:(q + 1) * B, d * H:(d + 1) * H] = o
        hlast = hl.reshape(P, KR, B).transpose(2, 1, 0).reshape(B, H)
        (hf if d == 0 else hb)[q * B:(q + 1) * B] = hlast
    return out, hf, hb


def kernel(x, params):
    T = 512
    x = np.asarray(x, np.float32)
    mode = MM_MODE

    layer_in = x
    finals = []
    for lp in params:
        out, hf, hb = _run_layer(layer_in, lp["fwd"], lp["bwd"], mode, T)
        finals.extend([hf, hb])
        layer_in = out
    hidden = np.stack(finals)
    return layer_in, hidden
